# revision 15
# baseline (speedup 1.0000x reference)
"""Disentangled MHA (DeBERTa-style) Trainium2 Bass kernel.

Sharding: 16 heads across 8 cores (2 heads/core), batch kept local.
Per core: project q/k/v with a 128-column weight slice, build the
relative-position score bands, skew-gather them via a DRAM round trip,
softmax (transposed orientation, unnormalized-exp + fused Z column),
and PV matmul. Host concatenates the per-core 128-feature outputs.

B=4, S=512, DIM=1024, H=16, HD=64, MAX_REL=512.
"""

import numpy as np

import concourse.bass as bass
import concourse.bacc as bacc
import concourse.mybir as mybir
import concourse.tile as tile
from concourse.bass_utils import run_bass_kernel_spmd
from concourse.masks import make_identity

B, S, DIM, H, HD = 4, 512, 1024, 16, 64
T = B * S                      # 2048 tokens
R = 1024                       # 2 * att_span rel rows
HC = 2                         # heads per core
NCORES = 8
KC = DIM // 128                # contraction chunks
SCALE = float((HD * 3) ** (-0.5))
BAND = 640                     # skew band width (needs >= 512 + 127)

F32 = mybir.dt.float32
F32R = mybir.dt.float32r
F16 = mybir.dt.float16
AF = mybir.ActivationFunctionType
ALU = mybir.AluOpType


def _r32(ap):
    return ap.bitcast(F32R)


def build_nc():
    nc = bacc.Bacc("TRN2", target_bir_lowering=False, debug=False)

    xT_d = nc.dram_tensor("xT", [DIM, T], F16, kind="ExternalInput")
    relT_d = nc.dram_tensor("relT", [DIM, R], F16, kind="ExternalInput")
    W_d = {
        n: nc.dram_tensor(f"W{n}", [DIM, 128], F16, kind="ExternalInput")
        for n in "qkv"
    }
    b_d = {
        n: nc.dram_tensor(f"b{n}", [128, 1], F32, kind="ExternalInput")
        for n in "qkv"
    }
    out_d = nc.dram_tensor("out", [T, 128], F32, kind="ExternalOutput")

    with tile.TileContext(nc) as tc:
        _body(nc, tc, xT_d.ap(), relT_d.ap(),
              {n: W_d[n].ap() for n in "qkv"},
              {n: b_d[n].ap() for n in "qkv"},
              out_d.ap())
    nc.compile()
    return nc


def _body(nc, tc, xT, relT, W, bvec, out_d):
    from contextlib import ExitStack
    ctx = ExitStack()
    with ctx:
        singles = ctx.enter_context(tc.tile_pool(name="singles", bufs=1))

        # ---- Load inputs ----
        xT_t = []
        for i in range(KC):
            t = singles.tile([128, T], F16, name=f"xT{i}")
            nc.sync.dma_start(out=t, in_=xT[i * 128:(i + 1) * 128, :])
            xT_t.append(t)
        relT_t = []
        for i in range(KC):
            t = singles.tile([128, R], F16, name=f"relT{i}")
            nc.sync.dma_start(out=t, in_=relT[i * 128:(i + 1) * 128, :])
            relT_t.append(t)
        W_t = {}
        for n in "qkv":
            W_t[n] = []
            for i in range(KC):
                t = singles.tile([128, 128], F16, name=f"W{n}{i}")
                nc.sync.dma_start(out=t, in_=W[n][i * 128:(i + 1) * 128, :])
                W_t[n].append(t)
        b_t = {}
        for n in "qkv":
            b_t[n] = singles.tile([128, 1], F32, name=f"b{n}")
            nc.sync.dma_start(out=b_t[n], in_=bvec[n])

        ident = singles.tile([128, 128], F16, name="ident")
        make_identity(nc, ident)

        # ---- Phase A: projections (fp32r matmuls, fp16 outputs) ----
        q2T = singles.tile([128, T], F16, name="q2T")
        k2T = singles.tile([128, T], F16, name="k2T")
        v2T = singles.tile([128, T], F16, name="v2T")
        posk = singles.tile([128, R], F16, name="posk")
        posq = singles.tile([128, R], F16, name="posq")

        projs = [
            (q2T, xT_t, "q", T),
            (k2T, xT_t, "k", T),
            (v2T, xT_t, "v", T),
            (posk, relT_t, "k", R),
            (posq, relT_t, "q", R),
        ]
        with tc.tile_pool(name="psA", space="PSUM", bufs=2) as psA:
            for out_sb, rhs_tiles, wn, n_tot in projs:
                for nt in range(n_tot // 512):
                    ps = psA.tile([128, 512], F32, name="ps_proj", tag="ps_proj")
                    for kc in range(KC):
                        nc.tensor.matmul(
                            out=ps,
                            lhsT=W_t[wn][kc][:, :],
                            rhs=rhs_tiles[kc][:, nt * 512:(nt + 1) * 512],
                            start=(kc == 0), stop=(kc == KC - 1),
                        )
                    # cast f32->f16 + per-partition bias add
                    nc.scalar.activation(
                        out=out_sb[:, nt * 512:(nt + 1) * 512], in_=ps,
                        func=AF.Identity, bias=b_t[wn], scale=1.0,
                    )

            # ---- v_tok: transpose v2T to token-major, augmented ones col ----
            vtok = []
            for t in range(T // 128):
                vt = singles.tile([128, 130], F16, name=f"vtok{t}")
                vtok.append(vt)
            with tc.tile_pool(name="psVT", space="PSUM", bufs=2) as psVT:
                for t in range(T // 128):
                    pst = psVT.tile([128, 128], F16, name="ps_vt", tag="ps_vt")
                    nc.tensor.transpose(pst, v2T[:, t * 128:(t + 1) * 128], ident)
                    nc.vector.tensor_copy(vtok[t][:, 0:64], pst[:, 0:64])
                    nc.vector.tensor_copy(vtok[t][:, 65:129], pst[:, 64:128])
                    nc.gpsimd.memset(vtok[t][:, 64:65], 1.0)
                    nc.gpsimd.memset(vtok[t][:, 129:130], 1.0)

        # ---- Phase B ----
        band_dram = ctx.enter_context(
            tc.tile_pool(name="bands", space="DRAM", bufs=64))
        sb_band = ctx.enter_context(tc.tile_pool(name="sb_band", bufs=4))
        sb_work = ctx.enter_context(tc.tile_pool(name="sb_work", bufs=3))
        sb_out = ctx.enter_context(tc.tile_pool(name="sb_out", bufs=4))
        ps_band_pool = ctx.enter_context(
            tc.tile_pool(name="psBand", space="PSUM", bufs=2))
        ps_qk_pool = ctx.enter_context(
            tc.tile_pool(name="psQK", space="PSUM", bufs=2))
        ps_pv_pool = ctx.enter_context(
            tc.tile_pool(name="psPV", space="PSUM", bufs=2))

        copy_flip = [0]

        def psum_to_sbuf_f16(dst, src):
            # alternate engines to balance DVE/ACT load
            if copy_flip[0] % 2 == 0:
                nc.vector.tensor_copy(dst, src)
            else:
                nc.scalar.copy(dst, src)
            copy_flip[0] += 1

        for b in range(B):
            # --- bands: matmul -> psum -> sbuf(f16) -> dram ---
            # c2p: one [512, 1024]-pitch dram buffer per h, single write DMA,
            # so each downstream transpose read waits on one DMA lane only.
            c2p_bd = {}         # h -> dram tile [512, 1024]
            p2c_bd = {}         # (h, kb) -> dram tile [128, BAND]
            for h in range(HC):
                hs = slice(h * 64, (h + 1) * 64)
                csb = sb_band.tile([128, 4 * BAND], F16,
                                   name="c2p_sb", tag="c2p_sb", bufs=2)
                for blk in range(4):
                    c0 = 128 * (3 - blk)
                    cs = slice(b * 512 + blk * 128, b * 512 + (blk + 1) * 128)
                    ps = ps_band_pool.tile([128, BAND], F32,
                                           name="ps_band", tag="ps_band")
                    nc.tensor.matmul(
                        out=ps[:, 0:512], lhsT=q2T[hs, cs],
                        rhs=posk[hs, c0:c0 + 512], start=True, stop=True)
                    nc.tensor.matmul(
                        out=ps[:, 512:BAND], lhsT=q2T[hs, cs],
                        rhs=posk[hs, c0 + 512:c0 + BAND], start=True, stop=True)
                    psum_to_sbuf_f16(csb[:, blk * BAND:(blk + 1) * BAND], ps)
                bdr = band_dram.tile([512, 1024], F16, name=f"c2pb_{b}{h}",
                                     tag="c2p_dram", bufs=4)
                # dram[qg*128+qi, 128*(3-qg)+j] = csb[qi, qg*640+j]
                dst = bass.AP(bdr.tensor, bdr.offset + 384,
                              [[1024, 128], [130944, 4], [1, BAND]])
                nc.sync.dma_start(
                    out=dst, in_=csb.rearrange("p (g j) -> p g j", g=4))
                c2p_bd[h] = bdr

                for blk in range(4):
                    c0 = 128 * (3 - blk)
                    cs = slice(b * 512 + blk * 128, b * 512 + (blk + 1) * 128)
                    ps = ps_band_pool.tile([128, BAND], F32,
                                           name="ps_band", tag="ps_band")
                    nc.tensor.matmul(
                        out=ps[:, 0:512], lhsT=k2T[hs, cs],
                        rhs=posq[hs, c0:c0 + 512], start=True, stop=True)
                    nc.tensor.matmul(
                        out=ps[:, 512:BAND], lhsT=k2T[hs, cs],
                        rhs=posq[hs, c0 + 512:c0 + BAND], start=True, stop=True)
                    bsb = sb_band.tile([128, BAND], F16,
                                       name="band_sb", tag="band_sb")
                    psum_to_sbuf_f16(bsb, ps)
                    bdr = band_dram.tile([128, BAND], F16,
                                         name=f"p2cb_{b}{h}{blk}",
                                         tag="p2c_dram", bufs=16)
                    nc.sync.dma_start(out=bdr, in_=bsb)
                    p2c_bd[(h, blk)] = bdr

            # --- attention ---
            ost_tiles = [
                sb_out.tile([128, 128], F32, name=f"ostage{qc}", tag=f"ostage{qc}")
                for qc in range(4)
            ]
            for h in range(HC):
                hs = slice(h * 64, (h + 1) * 64)
                ps_pv = ps_pv_pool.tile([128, 260], F32, name="ps_pv", tag="ps_pv")
                eT_tiles = []
                for kb in range(4):
                    ks = slice(b * 512 + kb * 128, b * 512 + (kb + 1) * 128)
                    # qkT: [k 128, q 512]
                    ps_qk = ps_qk_pool.tile([128, 512], F32,
                                            name="ps_qk", tag="ps_qk")
                    nc.tensor.matmul(
                        out=ps_qk, lhsT=k2T[hs, ks],
                        rhs=q2T[hs, b * 512:(b + 1) * 512],
                        start=True, stop=True)

                    # c2pT: one transposed skew read over the full-pitch band
                    t_sb = sb_work.tile([128, 512], F16, name="t_sb", tag="t_sb")
                    bdr = c2p_bd[h]
                    src = bass.AP(bdr.tensor, bdr.offset + 512 + 128 * kb,
                                  [[1023, 512], [1, 128]])
                    nc.sync.dma_start_transpose(out=t_sb, in_=src)
                    # p2cT: accumulate plain skew read
                    bdr = p2c_bd[(h, kb)]
                    src = bass.AP(bdr.tensor, bdr.offset + 128,
                                  [[BAND - 1, 128], [1, 512]])
                    nc.gpsimd.dma_start(out=t_sb, in_=src, accum_op=ALU.add)

                    # scores + exp (unnormalized, transposed): eT = exp(scale*s)
                    s_sb = sb_work.tile([128, 512], F16, name="s_sb", tag="s_sb")
                    nc.vector.tensor_tensor(out=s_sb, in0=t_sb, in1=ps_qk,
                                            op=ALU.add)
                    eT = sb_work.tile([128, 512], F16, name="eT", tag="eT",
                                      bufs=6)
                    nc.scalar.activation(out=eT, in_=s_sb, func=AF.Exp,
                                         scale=SCALE)
                    eT_tiles.append(eT)

                # PV (+ fused Z in col 64 of each 65-wide group); qc-outer so
                # each psum accumulation group closes before the next opens
                for qc in range(4):
                    for kb in range(4):
                        nc.tensor.matmul(
                            out=ps_pv[:, qc * 65:(qc + 1) * 65],
                            lhsT=eT_tiles[kb][:, qc * 128:(qc + 1) * 128],
                            rhs=vtok[b * 4 + kb][:, h * 65:h * 65 + 65],
                            start=(kb == 0), stop=(kb == 3))

                # --- finalize: divide by Z, stage, write out ---
                for qc in range(4):
                    zrec = sb_work.tile([128, 1], F32, name="zrec", tag="zrec")
                    nc.vector.reciprocal(zrec, ps_pv[:, qc * 65 + 64:qc * 65 + 65])
                    ost = ost_tiles[qc]
                    nc.scalar.activation(
                        out=ost[:, h * 64:(h + 1) * 64],
                        in_=ps_pv[:, qc * 65:qc * 65 + 64],
                        func=AF.Copy, scale=zrec)
                    if h == HC - 1:
                        rows = slice(b * 512 + qc * 128, b * 512 + (qc + 1) * 128)
                        nc.sync.dma_start(out=out_d[rows, :], in_=ost)


_NC_CACHE = None


def _get_nc():
    global _NC_CACHE
    if _NC_CACHE is None:
        _NC_CACHE = build_nc()
    return _NC_CACHE


def make_in_maps(inputs):
    x = np.asarray(inputs["x"], np.float32)
    rel = np.asarray(inputs["rel_embeddings"], np.float32)
    Wq = np.asarray(inputs["Wq"], np.float32)
    Wk = np.asarray(inputs["Wk"], np.float32)
    Wv = np.asarray(inputs["Wv"], np.float32)
    bq = np.asarray(inputs["bq"], np.float32)
    bk = np.asarray(inputs["bk"], np.float32)
    bv = np.asarray(inputs["bv"], np.float32)

    xT = np.ascontiguousarray(x.reshape(T, DIM).T).astype(np.float16)
    relT = np.ascontiguousarray(rel[::-1].T).astype(np.float16)
    in_maps = []
    for c in range(NCORES):
        sl = slice(c * 128, (c + 1) * 128)
        in_maps.append({
            "xT": xT,
            "relT": relT,
            "Wq": np.ascontiguousarray(Wq[:, sl]).astype(np.float16),
            "Wk": np.ascontiguousarray(Wk[:, sl]).astype(np.float16),
            "Wv": np.ascontiguousarray(Wv[:, sl]).astype(np.float16),
            "bq": np.ascontiguousarray(bq[sl]).reshape(128, 1),
            "bk": np.ascontiguousarray(bk[sl]).reshape(128, 1),
            "bv": np.ascontiguousarray(bv[sl]).reshape(128, 1),
        })
    return in_maps


def kernel(**inputs):
    nc = _get_nc()
    in_maps = make_in_maps(inputs)
    res = run_bass_kernel_spmd(nc, in_maps, list(range(NCORES))).results
    out = np.concatenate([res[c]["out"] for c in range(NCORES)], axis=1)
    return out.reshape(B, S, DIM).astype(np.float32)
